# revision 57
# baseline (speedup 1.0000x reference)
"""Trainium2 Bass kernel for nn_DotMatrix.

Math: for each (b, ell, t) the reference computes a complex pairwise dot
matrix O[i,j] = sum_m z[i,m] * w[j,m] where z = rep[b,:,t,:,:] as complex
and w the sign-flipped conjugation partner.  As a real matmul:

  lhsT[k, i]   k = (c,m) stacked: [Zr.T; Zi.T]                 [2m, 256]
  rhs[k, 2j+c'] c'=0: [FZr; -FZi], c'=1: [FZi; FZr]            [2m, 512]
  out = lhsT.T @ rhs  -> [256 i, 512 (j,c)]

with FZr[m',j] = s[m'] * Zr[j, M-1-m'], s[m'] = (-1)^(ell+m').

Precision trick: fp32 matmuls run at 4 cycles/column on the PE, but the
contraction dim here is tiny (2m <= 14), so we decompose each operand
into three bf16 parts (hi/mid/lo, 24 mantissa bits total) and stack the
six significant cross terms along the dead K dimension:

  L = [Ah; Am; Al; Ah; Am; Ah]   R = [Bh; Bh; Bh; Bm; Bm; Bl]

One bf16 matmul (K = 6*2m <= 84) then equals the fp32 product to
~2^-24, at 1 cycle/column — 4x faster than the fp32 path and with fast
(FWL) weight loads.

Symmetry trick: the pairwise matrix is symmetric in (i,j) for both the
real and imaginary components (O[i,j] = O[j,i]), so each channel only
computes 32-row i-blocks against j >= 32*bi — 56.25% of the matrix —
and the host mirrors the lower block-triangle for free.

Sharding: 8 cores = 2 batches x 4 tau-quarters.  Each core owns 32
channels ch = ell*8 + s (t = tq*8 + s).  Four channels (a quad) share
each matmul's 128 PSUM partitions via column tiling (tile_position),
the PSUM is evacuated by alternating ScalarE/VectorE copies into a
[128, 2304] staging tile, and each quad leaves as one contiguous
1.18MB HWDGE store.  Inputs are partition-packed into one full-height
[128, 12288] tensor (ell3+ell0 rows 0:84/96:108, ell2+ell1 rows
0:60/64:100 — matmul base partitions are restricted to 0/32/64/96)
loaded as three large full-width chunks so input DMA uses all 16
SBUF ports with minimal descriptor-emission overhead, and a short
dependency-free dummy-matmul chain pre-warms the PE clock gate.
Host reassembles the full [2,256,256,128,2] output.
"""

import numpy as np
import ml_dtypes

import concourse.bass as bass
import concourse.bacc as bacc
import concourse.mybir as mybir
from concourse.bass_utils import run_bass_kernel_spmd
from concourse.tile import TileContext

B, N, TAU, NELL = 2, 256, 32, 4
NCORES = 8
NCH = 32          # channels per core (4 ell * 8 slots)
F32 = mybir.dt.float32
BF16 = mybir.dt.bfloat16
BFNP = ml_dtypes.bfloat16
KS = [6 * 2 * (2 * ell + 1) for ell in range(NELL)]   # 12, 36, 60, 84
BIW = [512 - 64 * bi for bi in range(8)]              # cols per 32-row i-block
BIO = [0, 512, 960, 1344, 1664, 1920, 2112, 2240]     # ot offsets per i-block
OTW = 2304                                            # sum(BIW)

_NC_CACHE = {}


def _build_bass():
    nc = bacc.Bacc()
    # Single packed input: cols [0:3072) = A slots 0-3 (critical first
    # chunk), [3072:6144) = A slots 4-7, [6144:12288) = B; rows 0:84 =
    # ell3, 96:108 = ell0 (A-cols), 0:60 = ell2, 64:100 = ell1 (B-cols).
    inp_d = nc.declare_dram_parameter("inp", [128, 12288], BF16, isOutput=False)
    # The pairwise matrix is symmetric in (i,j), so each channel only
    # computes i-blocks of 32 against j >= 32*bi (block upper triangle,
    # 56.25% of the full matrix); the host mirrors the rest.  Four channels
    # (a quad) share each matmul's 128 PSUM partitions via column tiling.
    # Layout: [quad, psum_row, (bi-block columns)] — contiguous per quad.
    out = nc.declare_dram_parameter("out", [NCH // 4, 128, OTW], F32, isOutput=True)

    with TileContext(nc) as tc:
        with (
            tc.tile_pool(name="lin", bufs=1) as lin_pool,
            tc.tile_pool(name="rin", bufs=1) as rin_pool,
            tc.tile_pool(name="ps", bufs=8, space="PSUM") as ps_pool,
            tc.tile_pool(name="ot", bufs=5) as ot_pool,
        ):
            in_sb = lin_pool.tile([128, 12288], BF16, name="in_sb")
            # PE pre-warm: dependency-free dummy matmuls on scratch tiles keep
            # the PE busy from kernel start, so the HAM clock gate is already
            # released (2.4 GHz) when the first real matmuls arrive.
            warm_in = lin_pool.tile([128, 512], BF16, name="warm_in")
            warm_ps = ps_pool.tile([128, 512], F32, tag="ps", name="warm_ps")
            nc.vector.memset(warm_in[:], 0.0)
            for _ in range(10):
                nc.tensor.matmul(
                    warm_ps[:], warm_in[:, 0:128], warm_in[:, 0:512],
                    start=True, stop=True,
                )
            # ell -> (packed tensor idx, base partition)
            pack = {3: (0, 0), 0: (0, 96), 2: (1, 0), 1: (1, 64)}
            # Input loads ride the sync HWDGE ring ahead of the output
            # stores (strict FIFO, single queue keeps full engine attention);
            # three large full-width chunks minimize descriptor-emission
            # overhead while the first chunk unblocks the A quads early.
            nc.sync.dma_start(out=in_sb[:, 0:3072], in_=inp_d[:, 0:3072])
            nc.sync.dma_start(out=in_sb[:, 3072:6144], in_=inp_d[:, 3072:6144])
            nc.sync.dma_start(out=in_sb[:, 6144:12288], in_=inp_d[:, 6144:12288])
            n_copy = 0
            quad_order = [(0, 0), (3, 0), (0, 1), (3, 1), (2, 0), (1, 0), (2, 1), (1, 1)]
            for e, v in quad_order:
                K = KS[e]
                t, bp = pack[e]
                ot = ot_pool.tile([128, OTW], F32)
                for bi in range(8):     # i-block of 32 rows
                    W = BIW[bi]
                    ps = ps_pool.tile([128, 512], F32)
                    for c4 in range(4):  # channel within quad
                        sl = v * 4 + c4
                        lo = _lhs_off(t, sl)
                        ro = _rhs_off(t, sl)
                        nc.tensor.matmul(
                            ps[c4 * 32 : (c4 + 1) * 32, 0:W],
                            in_sb[
                                bp : bp + K,
                                lo + bi * 32 : lo + bi * 32 + 32,
                            ],
                            in_sb[
                                bp : bp + K, ro + 64 * bi : ro + 512
                            ],
                            start=True,
                            stop=True,
                            tile_position=(bp, c4 * 32),
                        )
                    dst = ot[:, BIO[bi] : BIO[bi] + W]
                    if n_copy % 2 == 0:
                        nc.scalar.copy(dst, ps[:, 0:W])
                    else:
                        nc.vector.tensor_copy(out=dst, in_=ps[:, 0:W])
                    n_copy += 1
                qidx = e * 2 + v
                # alternate stores across the two physical HWDGE rings so
                # SDMA engines can pull from the other ring at store
                # boundaries instead of idling
                eng = nc.sync if qidx % 2 == 0 else nc.scalar
                eng.dma_start(out=out[qidx], in_=ot[:])
    nc.compile()
    return nc


def _dec3(x):
    h = x.astype(BFNP)
    r = x - h.astype(np.float32)
    m_ = r.astype(BFNP)
    l = (r - m_.astype(np.float32)).astype(BFNP)
    return h, m_, l


_PACK = {3: (0, 0), 0: (0, 96), 2: (1, 0), 1: (1, 64)}


def _lhs_off(t, sl):
    # column of slot sl's lhsT block inside the packed input tile
    if t == 0:
        return sl * 256 if sl < 4 else 3072 + (sl - 4) * 256
    return 6144 + sl * 256


def _rhs_off(t, sl):
    if t == 0:
        return 1024 + sl * 512 if sl < 4 else 4096 + (sl - 4) * 512
    return 8192 + sl * 512


def _host_prep(reps, cid):
    """Build per-core bf16 K-stacked lhs/rhs tensors (partition-packed)."""
    b, tq = cid // 4, cid % 4
    im = {"inp": np.zeros((128, 12288), BFNP)}
    for ell in range(NELL):
        rep = reps[ell]
        m = 2 * ell + 1
        s_vec = ((-1.0) ** (ell + np.arange(m))).astype(np.float32)
        tp_, bp = _PACK[ell]
        INP = im["inp"]
        for sidx in range(8):
            t = tq * 8 + sidx
            Z = rep[b, :, t]                      # [256, m, 2]
            Zr, Zi = Z[..., 0], Z[..., 1]         # [256, m]
            lhsT = np.concatenate([Zr.T, Zi.T], axis=0)      # [2m, 256]
            FZr = s_vec[:, None] * Zr[:, ::-1].T             # [m, 256]
            FZi = s_vec[:, None] * Zi[:, ::-1].T
            R = np.empty((2 * m, 256, 2), np.float32)
            R[0:m, :, 0] = FZr
            R[m:, :, 0] = -FZi
            R[0:m, :, 1] = FZi
            R[m:, :, 1] = FZr
            rhs = R.reshape(2 * m, 512)
            Ah, Am, Al = _dec3(lhsT)
            Bh, Bm, Bl = _dec3(rhs)
            lo = _lhs_off(tp_, sidx)
            ro = _rhs_off(tp_, sidx)
            INP[bp : bp + KS[ell], lo : lo + 256] = np.concatenate(
                [Ah, Am, Al, Ah, Am, Ah], axis=0
            )
            INP[bp : bp + KS[ell], ro : ro + 512] = np.concatenate(
                [Bh, Bh, Bh, Bm, Bm, Bl], axis=0
            )
    return im


def _run(in_maps, **kw):
    if "nc" not in _NC_CACHE:
        _NC_CACHE["nc"] = _build_bass()
    return run_bass_kernel_spmd(_NC_CACHE["nc"], in_maps, list(range(NCORES)), **kw)


def kernel(rep0, rep1, rep2, rep3, _bass_kw=None):
    reps = [np.ascontiguousarray(np.asarray(r, dtype=np.float32)) for r in (rep0, rep1, rep2, rep3)]
    in_maps = [_host_prep(reps, cid) for cid in range(NCORES)]
    res = _run(in_maps, **(_bass_kw or {}))
    out = np.empty((B, N, N, NELL * TAU, 2), np.float32)
    for cid in range(NCORES):
        b, tq = cid // 4, cid % 4
        arr = res.results[cid]["out"]          # [8, 128, OTW]
        o = np.empty((NCH, 256, 256, 2), np.float32)
        for bi in range(8):
            nj = 256 - 32 * bi
            blk = arr[:, :, BIO[bi] : BIO[bi] + BIW[bi]].reshape(
                NCH // 4, 4, 32, nj, 2
            )
            # blk[quad, c4, i_in_block, j - 32*bi, c]; ch = 4*quad + c4
            for c4 in range(4):
                o[c4::4, 32 * bi : 32 * bi + 32, 32 * bi :, :] = blk[:, c4]
        for bi in range(1, 8):                  # mirror lower block triangle
            r = slice(32 * bi, 32 * bi + 32)
            o[:, r, : 32 * bi, :] = o[:, : 32 * bi, r, :].transpose(0, 2, 1, 3)
        for ell in range(NELL):
            lo = ell * TAU + tq * 8
            out[b, :, :, lo : lo + 8, :] = o[ell * 8 : (ell + 1) * 8].transpose(
                1, 2, 0, 3
            )
    kernel.last_result = res
    return out
